# revision 17
# baseline (speedup 1.0000x reference)
"""Trainium2 Bass kernel for nn_CliffordLinearEquivariant.

Math: the reference folds both geometric products and both weight tensors
into a tiny T[o,i,q,r] tensor, then does one big memory-bound contraction:

    out[b,s,o,r] = sum_{i,q} T[o,i,q,r] * x[b,s,i,q] + bias[o,r]

Flattening (i,q)->128 and (o,r)->128 this is a plain GEMM over tokens:

    out[tok, 128] = x[tok, 128] @ T2[128, 128] + bias[128]

with tok = B*S = 262144. We shard tokens 8 ways (data parallel), fold the
tiny weights into T2 on host (float64, then cast), and run a Bass/Tile
kernel per core.

The contraction is memory-bound, so the kernel runs in float16: x is cast
to fp16 on host (halves HBM read traffic AND doubles PE throughput: fp16
matmul/transpose run 1 cycle/row vs 4/2 for fp32), the matmul accumulates
in fp32 PSUM, and the output is written as fp16 (halves HBM write traffic)
then upcast to fp32 on host. Max rel err of the fp16 pipeline measured at
5e-4 against the fp64 reference -- 37x inside the 2e-2 gate.

Per core: DMA x in 1MB fp16 chunks (8KB contiguous per-partition lines) ->
PE transpose 128x128 token blocks (contraction dim onto partitions) -> ACT
copies the transposed block PSUM->SBUF (fp16) -> PE matmul against resident
T2 -> DVE adds bias during the mandatory PSUM->SBUF copy (fp16 out) -> DMA
out. The two PSUM->SBUF traversals are split across ACT and DVE so neither
engine exceeds the DMA time.
"""
import sys

sys.path.insert(0, "/opt/trn_rl_repo")

import numpy as np

_DIM = 8
_B, _S, _I, _O, _K = 64, 4096, 16, 16, 2
_NCORES = 8
_NTOK = _B * _S
_TOK = _NTOK // _NCORES       # tokens per core
_CH = 4096                    # tokens per DMA chunk (1 MiB fp16)
_GRP = 1024                   # tokens per PSUM group (2 banks)

_cache = {}


def _cayley():
    C = np.zeros((_DIM, _DIM, _DIM), dtype=np.float64)
    metric = np.array([1.0, 1.0, 1.0])
    for a in range(_DIM):
        for b in range(_DIM):
            s, aa = 0, a >> 1
            while aa:
                s += bin(aa & b).count("1")
                aa >>= 1
            sign = -1.0 if (s & 1) else 1.0
            common = a & b
            for i in range(3):
                if common & (1 << i):
                    sign *= metric[i]
            C[a, b, a ^ b] = sign
    return C


def _fold_weights(weight_left, weight_right):
    """T2[(i,q),(o,r)] with T[o,i,q,r] = sum_{k,p,m,s} wl C C wr."""
    C = _cayley()
    wl = weight_left.astype(np.float64)
    wr = weight_right.astype(np.float64)
    A = np.einsum("koip,pqm->koiqm", wl, C)
    Bm = np.einsum("kois,msr->koimr", wr, C)
    T = np.einsum("koiqm,koimr->oiqr", A, Bm)          # [O, I, 8, 8]
    T2 = T.transpose(1, 2, 0, 3).reshape(_I * _DIM, _O * _DIM)
    return np.ascontiguousarray(T2, dtype=np.float16)


def _build_nc(TOK=_TOK, CH=_CH, GRP=_GRP, ps_t_bufs=3, ps_o_bufs=2,
              xt_bufs=4, sb_bufs=4, store_engine="act",
              upfront_loads=False):
    """fp16 data path, fp32 PSUM accumulate.

    GRP = tokens per PSUM group; [128, GRP] PSUM tiles. All chunk loads
    are issued up front (xin pool holds the whole shard) so the SP
    sequencer never has a store wait blocking a later load; stores
    follow in program order and each waits only on its own otile. The
    identity for PE transposes ships from HBM so GPSIMD (and its ucode
    table loads) stays out of the instruction stream entirely.
    """
    import concourse.bacc as bacc
    import concourse.mybir as mybir
    from concourse.tile import TileContext

    F32 = mybir.dt.float32
    F16 = mybir.dt.float16
    NB = CH // 128
    nch = TOK // CH
    nblk = GRP // 128          # 128-token blocks per group
    nc = bacc.Bacc("TRN2")
    xs = nc.dram_tensor("xs", [TOK, 128], F16, kind="ExternalInput")
    t2 = nc.dram_tensor("t2", [128, 128], F16, kind="ExternalInput")
    idn = nc.dram_tensor("idn", [128, 128], F16, kind="ExternalInput")
    bb4 = nc.dram_tensor("bb4", [128, GRP], F32, kind="ExternalInput")
    out = nc.dram_tensor("out", [TOK, 128], F16, kind="ExternalOutput")

    # Contiguous-per-partition layout: partition p of chunk c holds NB
    # consecutive tokens, so each DMA line is one contiguous 2*128*NB-byte
    # run (measured ~4.4x faster than interleaving tokens across
    # partitions, which produced short strided runs). The token->
    # partition permutation is identical for loads and stores, so
    # correctness is unaffected.
    x_view = xs.rearrange("(c p b) f -> c p (b f)", p=128, b=NB)
    o_view = out.rearrange("(c p b) f -> c p (b f)", p=128, b=NB)

    with TileContext(nc) as tc:
        with (
            tc.tile_pool(name="const", bufs=1) as cpool,
            tc.tile_pool(name="xin", bufs=nch if upfront_loads else sb_bufs) as xpool,
            tc.tile_pool(name="xt", bufs=xt_bufs) as xtpool,
            tc.tile_pool(name="outp", bufs=3) as opool,
            tc.tile_pool(name="ps_t", bufs=ps_t_bufs, space="PSUM") as pst,
            tc.tile_pool(name="ps_o", bufs=ps_o_bufs, space="PSUM") as pso,
        ):
            t2_s = cpool.tile([128, 128], F16)
            nc.sync.dma_start(t2_s, t2[:, :])
            bb_s = cpool.tile([128, GRP], F32)
            nc.sync.dma_start(bb_s, bb4[:, :])
            ident = cpool.tile([128, 128], F16)
            nc.sync.dma_start(ident, idn[:, :])

            xtiles = {}
            if upfront_loads:
                for c in range(nch):
                    xtile = xpool.tile([128, CH], F16)
                    nc.sync.dma_start(xtile, x_view[c])
                    xtiles[c] = xtile

            # Warm each engine's vector clock on every constant so
            # steady-state instructions carry at most one sync wait
            # (HW instruction structs have a single wait slot).
            scratch_t = pst.tile([128, GRP], F16, tag="xt_ps")
            scratch_o = pso.tile([128, GRP], F32, tag="o_ps")
            scratch_sb = cpool.tile([128, GRP], F16)
            nc.tensor.transpose(scratch_t[:, :128], ident, ident)
            nc.scalar.copy(scratch_sb, scratch_t)
            nc.tensor.matmul(scratch_o[:, :128], ident, t2_s)
            nc.vector.tensor_add(scratch_sb, scratch_o, bb_s)

            for c in range(nch):
                if upfront_loads:
                    xtile = xtiles[c]
                else:
                    xtile = xpool.tile([128, CH], F16)
                    nc.sync.dma_start(xtile, x_view[c])
                otile = opool.tile([128, CH], F16)
                for g in range(CH // GRP):
                    xt_ps = pst.tile([128, GRP], F16, tag="xt_ps")
                    for b in range(nblk):
                        blk = g * nblk + b
                        nc.tensor.transpose(
                            xt_ps[:, b * 128:(b + 1) * 128],
                            xtile[:, blk * 128:(blk + 1) * 128],
                            ident,
                        )
                    xt_sb = xtpool.tile([128, GRP], F16)
                    nc.scalar.copy(xt_sb, xt_ps)
                    o_ps = pso.tile([128, GRP], F32, tag="o_ps")
                    for b in range(nblk):
                        nc.tensor.matmul(
                            o_ps[:, b * 128:(b + 1) * 128],
                            xt_sb[:, b * 128:(b + 1) * 128],
                            t2_s,
                        )
                    nc.vector.tensor_add(
                        otile[:, g * GRP:(g + 1) * GRP], o_ps, bb_s
                    )
                # Stores issue from their own engine queue so a store
                # waiting on compute can't head-of-line-block the next
                # chunk load behind it in the SP sequencer.
                if store_engine == "gpsimd":
                    nc.gpsimd.dma_start(o_view[c], otile)
                elif store_engine == "act":
                    nc.scalar.dma_start(o_view[c], otile)
                elif store_engine == "vector":
                    nc.vector.dma_start(o_view[c], otile)
                else:
                    nc.sync.dma_start(o_view[c], otile)
    nc.compile()
    return nc


def _get_runner():
    """Build (once) a jitted shard_map callable over the 8-core mesh."""
    if "runner" in _cache:
        return _cache["runner"]

    import jax
    from jax.sharding import Mesh, PartitionSpec
    from jax.experimental.shard_map import shard_map
    import concourse.mybir as mybir
    from concourse import bass2jax

    bass2jax.install_neuronx_cc_hook()
    nc = _build_nc()
    _cache["nc"] = nc

    partition_name = (
        nc.partition_id_tensor.name if nc.partition_id_tensor else None
    )
    in_names = []
    out_names = []
    out_avals = []
    for alloc in nc.m.functions[0].allocations:
        if not isinstance(alloc, mybir.MemoryLocationSet):
            continue
        name = alloc.memorylocations[0].name
        if alloc.kind == "ExternalInput":
            if name != partition_name:
                in_names.append(name)
        elif alloc.kind == "ExternalOutput":
            out_names.append(name)
            out_avals.append(
                jax.core.ShapedArray(
                    tuple(alloc.tensor_shape), mybir.dt.np(alloc.dtype)
                )
            )
    n_params = len(in_names)
    all_in_names = in_names + out_names
    if partition_name is not None:
        all_in_names = all_in_names + [partition_name]

    def _body(*args):
        operands = list(args)
        if partition_name is not None:
            operands.append(bass2jax.partition_id_tensor())
        outs = bass2jax._bass_exec_p.bind(
            *operands,
            out_avals=tuple(out_avals),
            in_names=tuple(all_in_names),
            out_names=tuple(out_names),
            lowering_input_output_aliases=(),
            sim_require_finite=True,
            sim_require_nnan=True,
            nc=nc,
        )
        return tuple(outs)

    devices = jax.devices()[:_NCORES]
    mesh = Mesh(np.asarray(devices), ("core",))
    spec = PartitionSpec("core")
    n_outs = len(out_names)
    # No donation: the kernel writes every output element, so the dummy
    # output operands (required by the bass custom call's parameter
    # list) can be one cached device-resident zeros array reused across
    # calls instead of a fresh 64MB host->device transfer per call.
    fn = jax.jit(
        shard_map(
            _body,
            mesh=mesh,
            in_specs=(spec,) * (n_params + n_outs),
            out_specs=(spec,) * n_outs,
            check_rep=False,
        ),
        keep_unused=True,
    )
    _cache["runner"] = (fn, in_names, out_names, mesh, spec)
    return _cache["runner"]


def _prepare_inputs(x, weight_left, weight_right, bias):
    """Host-side prep: shard x, fold weights, broadcast bias."""
    T2 = _fold_weights(weight_left, weight_right)
    bias_flat = np.ascontiguousarray(bias, dtype=np.float32).reshape(_O * _DIM)
    BB4 = np.tile(
        np.broadcast_to(bias_flat, (128, 128)), (1, _GRP // 128)
    ).astype(np.float32)
    x_flat = np.ascontiguousarray(x, dtype=np.float32).reshape(_NTOK, 128)
    x_flat = x_flat.astype(np.float16)
    # global concat layout for shard_map: inputs stacked along axis 0
    ins = {
        "xs": x_flat,                                   # [NTOK, 128] fp16
        "t2": np.tile(T2, (_NCORES, 1)),                # replicate per core
        "idn": np.tile(np.eye(128, dtype=np.float16), (_NCORES, 1)),
        "bb4": np.tile(BB4, (_NCORES, 1)),
    }
    return ins


def _out_dummy():
    """Cached device-resident dummy for the output operand slot."""
    import jax
    from jax.sharding import NamedSharding

    if "zeros" not in _cache:
        fn, in_names, out_names, mesh, spec = _get_runner()
        sharding = NamedSharding(mesh, spec)
        z = jax.device_put(np.zeros((_NTOK, 128), np.float16), sharding)
        z.block_until_ready()
        _cache["zeros"] = z
    return _cache["zeros"]


def _run_device(ins):
    import jax
    from jax.sharding import NamedSharding

    fn, in_names, out_names, mesh, spec = _get_runner()
    sharding = NamedSharding(mesh, spec)
    args = [jax.device_put(ins[n], sharding) for n in in_names]
    outs = fn(*args, _out_dummy())
    return np.asarray(outs[0])


def kernel(x, weight_left, weight_right, bias):
    x = np.asarray(x)
    weight_left = np.asarray(weight_left)
    weight_right = np.asarray(weight_right)
    bias = np.asarray(bias)
    ins = _prepare_inputs(x, weight_left, weight_right, bias)
    out_flat = _run_device(ins)
    return out_flat.astype(np.float32).reshape(_B, _S, _O, _DIM)


def _bench_args():
    """Device-resident inputs for timing runs."""
    import jax
    from jax.sharding import NamedSharding

    rng = np.random.default_rng(0)
    x = rng.standard_normal((_B, _S, _I, _DIM), dtype=np.float32)
    wl = (rng.standard_normal((_K, _O, _I, _DIM)) * 0.02).astype(np.float32)
    wr = (rng.standard_normal((_K, _O, _I, _DIM)) * 0.02).astype(np.float32)
    bias = np.zeros((_O, _DIM), np.float32)
    ins = _prepare_inputs(x, wl, wr, bias)

    fn, in_names, out_names, mesh, spec = _get_runner()
    sharding = NamedSharding(mesh, spec)
    args = [jax.device_put(ins[n], sharding) for n in in_names]
    return fn, args


def _ntff_profile_hook():
    """NTFF profiling via ctypes into libaxon_pjrt.so (the NRT profile
    sidechannel). Returns a contextmanager factory or None if absent."""
    import contextlib
    import ctypes

    so_path = "/opt/axon/libaxon_pjrt.so"
    try:
        lib = ctypes.CDLL(so_path)
        lib.axon_start_nrt_profile
    except (OSError, AttributeError):
        return None
    lib.axon_start_nrt_profile.argtypes = [
        ctypes.POINTER(ctypes.c_int64), ctypes.c_size_t,
    ]
    lib.axon_start_nrt_profile.restype = ctypes.c_int64
    lib.axon_stop_nrt_profile.argtypes = [ctypes.c_char_p]
    lib.axon_stop_nrt_profile.restype = ctypes.c_int64

    @contextlib.contextmanager
    def _hook(output_dir, device_ids):
        import jax

        jax.devices()
        ids = (ctypes.c_int64 * len(device_ids))(*device_ids)
        rc = lib.axon_start_nrt_profile(ids, len(device_ids))
        if rc != 0:
            raise RuntimeError(f"axon_start_nrt_profile rc={rc}")
        try:
            yield
        finally:
            n = lib.axon_stop_nrt_profile(str(output_dir).encode())
            if n <= 0:
                raise RuntimeError(f"axon_stop_nrt_profile wrote {n} files")

    return _hook


def _hw_exec_time_profiled(n_reps=3):
    """True on-device execution time via neuron-profile (NTFF capture on
    all 8 cores; per capture take the max across cores = the parallel
    HW execution time; report the median across captures)."""
    import os
    import shutil
    import statistics

    hook = _ntff_profile_hook()
    if hook is None:
        return None
    from concourse._compat import FishPath
    import gauge.profiler

    fn, args = _bench_args()
    z = _out_dummy()
    fn(*args, z)[0].block_until_ready()  # warm

    nc = _cache["nc"]
    cores = list(range(_NCORES))
    per_capture_max = []
    for rep in range(n_reps):
        out_dir = f"/tmp/kernel_ntff_{os.getpid()}_{rep}"
        shutil.rmtree(out_dir, ignore_errors=True)
        os.makedirs(out_dir, exist_ok=True)
        with hook(out_dir, cores):
            fn(*args, z)[0].block_until_ready()
        profile = gauge.profiler.Profile(
            profile_path=FishPath(out_dir),
            kernel_dev_mode=True,
            profile_on_exit=False,
            bass_kernel=nc.m,
            offline_processing=True,
            fname="*_body*",
        )
        results = profile.to_perfetto(model_index=tuple(cores))
        times = [r.exec_time_ns for r in results if r.exec_time_ns]
        if not times:
            return None
        per_capture_max.append(max(times))
        shutil.rmtree(out_dir, ignore_errors=True)
    return statistics.median(per_capture_max)


def _timed_run(n_iters=30):
    """HW execution time in ns. Prefers the neuron-profile (NTFF) number
    — the true on-device time; falls back to host wall-clock around the
    device-resident-input launch (which on axon-tunneled cores is
    dominated by dispatch latency) if profiling is unavailable."""
    try:
        ns = _hw_exec_time_profiled()
        if ns is not None:
            return ns
    except Exception as e:  # noqa: BLE001 -- any profiling failure
        import sys as _sys

        print(f"ntff profiling unavailable ({e!r}); wall-clock fallback",
              file=_sys.stderr)

    import time

    fn, args = _bench_args()
    z = _out_dummy()
    fn(*args, z)[0].block_until_ready()  # compile+warm
    best = float("inf")
    for _ in range(n_iters):
        t0 = time.perf_counter()
        out = fn(*args, z)
        out[0].block_until_ready()
        best = min(best, time.perf_counter() - t0)
    return best * 1e9


if __name__ == "__main__":
    ns = _timed_run()
    print(f"HW exec time: {ns:.0f} ns")


# revision 19
# speedup vs baseline: 1.0149x; 1.0149x over previous
"""Trainium2 Bass kernel for nn_CliffordLinearEquivariant.

Math: the reference folds both geometric products and both weight tensors
into a tiny T[o,i,q,r] tensor, then does one big memory-bound contraction:

    out[b,s,o,r] = sum_{i,q} T[o,i,q,r] * x[b,s,i,q] + bias[o,r]

Flattening (i,q)->128 and (o,r)->128 this is a plain GEMM over tokens:

    out[tok, 128] = x[tok, 128] @ T2[128, 128] + bias[128]

with tok = B*S = 262144. We shard tokens 8 ways (data parallel), fold the
tiny weights into T2 on host (float64, then cast), and run a Bass/Tile
kernel per core.

The contraction is memory-bound, so the kernel runs in float16: x is cast
to fp16 on host (halves HBM read traffic AND doubles PE throughput: fp16
matmul/transpose run 1 cycle/row vs 4/2 for fp32), the matmul accumulates
in fp32 PSUM, and the output is written as fp16 (halves HBM write traffic)
then upcast to fp32 on host. Max rel err of the fp16 pipeline measured at
5e-4 against the fp64 reference -- 37x inside the 2e-2 gate.

Per core: DMA x in 1MB fp16 chunks (8KB contiguous per-partition lines) ->
PE transpose 128x128 token blocks (contraction dim onto partitions) -> ACT
copies the transposed block PSUM->SBUF (fp16) -> PE matmul against resident
T2 -> DVE adds bias during the mandatory PSUM->SBUF copy (fp16 out) -> DMA
out. The two PSUM->SBUF traversals are split across ACT and DVE so neither
engine exceeds the DMA time.
"""
import sys

sys.path.insert(0, "/opt/trn_rl_repo")

import numpy as np

_DIM = 8
_B, _S, _I, _O, _K = 64, 4096, 16, 16, 2
_NCORES = 8
_NTOK = _B * _S
_TOK = _NTOK // _NCORES       # tokens per core
_CH = 4096                    # tokens per DMA chunk (1 MiB fp16)
_GRP = 512                    # tokens per PSUM group (1 fp32 bank)

_cache = {}


def _cayley():
    C = np.zeros((_DIM, _DIM, _DIM), dtype=np.float64)
    metric = np.array([1.0, 1.0, 1.0])
    for a in range(_DIM):
        for b in range(_DIM):
            s, aa = 0, a >> 1
            while aa:
                s += bin(aa & b).count("1")
                aa >>= 1
            sign = -1.0 if (s & 1) else 1.0
            common = a & b
            for i in range(3):
                if common & (1 << i):
                    sign *= metric[i]
            C[a, b, a ^ b] = sign
    return C


def _fold_weights(weight_left, weight_right):
    """T2[(i,q),(o,r)] with T[o,i,q,r] = sum_{k,p,m,s} wl C C wr."""
    C = _cayley()
    wl = weight_left.astype(np.float64)
    wr = weight_right.astype(np.float64)
    A = np.einsum("koip,pqm->koiqm", wl, C)
    Bm = np.einsum("kois,msr->koimr", wr, C)
    T = np.einsum("koiqm,koimr->oiqr", A, Bm)          # [O, I, 8, 8]
    T2 = T.transpose(1, 2, 0, 3).reshape(_I * _DIM, _O * _DIM)
    return np.ascontiguousarray(T2, dtype=np.float16)


def _build_nc(TOK=_TOK, CH=_CH, GRP=_GRP, ps_t_bufs=4, ps_o_bufs=4,
              xt_bufs=8, sb_bufs=5, store_engine="act",
              upfront_loads=False):
    """fp16 data path, fp32 PSUM accumulate.

    GRP = tokens per PSUM group; [128, GRP] PSUM tiles. All chunk loads
    are issued up front (xin pool holds the whole shard) so the SP
    sequencer never has a store wait blocking a later load; stores
    follow in program order and each waits only on its own otile. The
    identity for PE transposes ships from HBM so GPSIMD (and its ucode
    table loads) stays out of the instruction stream entirely.
    """
    import concourse.bacc as bacc
    import concourse.mybir as mybir
    from concourse.tile import TileContext

    F32 = mybir.dt.float32
    F16 = mybir.dt.float16
    NB = CH // 128
    nch = TOK // CH
    nblk = GRP // 128          # 128-token blocks per group
    nc = bacc.Bacc("TRN2")
    xs = nc.dram_tensor("xs", [TOK, 128], F16, kind="ExternalInput")
    t2 = nc.dram_tensor("t2", [128, 128], F16, kind="ExternalInput")
    idn = nc.dram_tensor("idn", [128, 128], F16, kind="ExternalInput")
    bb4 = nc.dram_tensor("bb4", [128, GRP], F32, kind="ExternalInput")
    out = nc.dram_tensor("out", [TOK, 128], F16, kind="ExternalOutput")

    # Contiguous-per-partition layout: partition p of chunk c holds NB
    # consecutive tokens, so each DMA line is one contiguous 2*128*NB-byte
    # run (measured ~4.4x faster than interleaving tokens across
    # partitions, which produced short strided runs). The token->
    # partition permutation is identical for loads and stores, so
    # correctness is unaffected.
    x_view = xs.rearrange("(c p b) f -> c p (b f)", p=128, b=NB)
    o_view = out.rearrange("(c p b) f -> c p (b f)", p=128, b=NB)

    with TileContext(nc) as tc:
        with (
            tc.tile_pool(name="const", bufs=1) as cpool,
            tc.tile_pool(name="xin", bufs=nch if upfront_loads else sb_bufs) as xpool,
            tc.tile_pool(name="xt", bufs=xt_bufs) as xtpool,
            tc.tile_pool(name="outp", bufs=3) as opool,
            tc.tile_pool(name="ps_t", bufs=ps_t_bufs, space="PSUM") as pst,
            tc.tile_pool(name="ps_o", bufs=ps_o_bufs, space="PSUM") as pso,
        ):
            t2_s = cpool.tile([128, 128], F16)
            nc.sync.dma_start(t2_s, t2[:, :])
            bb_s = cpool.tile([128, GRP], F32)
            nc.sync.dma_start(bb_s, bb4[:, :])
            ident = cpool.tile([128, 128], F16)
            nc.sync.dma_start(ident, idn[:, :])

            xtiles = {}
            if upfront_loads:
                for c in range(nch):
                    xtile = xpool.tile([128, CH], F16)
                    nc.sync.dma_start(xtile, x_view[c])
                    xtiles[c] = xtile

            # Warm each engine's vector clock on every constant so
            # steady-state instructions carry at most one sync wait
            # (HW instruction structs have a single wait slot).
            scratch_t = pst.tile([128, GRP], F16, tag="xt_ps")
            scratch_o = pso.tile([128, GRP], F32, tag="o_ps")
            scratch_sb = cpool.tile([128, GRP], F16)
            nc.tensor.transpose(scratch_t[:, :128], ident, ident)
            nc.scalar.copy(scratch_sb, scratch_t)
            nc.tensor.matmul(scratch_o[:, :128], ident, t2_s)
            nc.vector.tensor_add(scratch_sb, scratch_o, bb_s)

            for c in range(nch):
                if upfront_loads:
                    xtile = xtiles[c]
                else:
                    xtile = xpool.tile([128, CH], F16)
                    nc.sync.dma_start(xtile, x_view[c])
                otile = opool.tile([128, CH], F16)
                for g in range(CH // GRP):
                    xt_ps = pst.tile([128, GRP], F16, tag="xt_ps")
                    for b in range(nblk):
                        blk = g * nblk + b
                        nc.tensor.transpose(
                            xt_ps[:, b * 128:(b + 1) * 128],
                            xtile[:, blk * 128:(blk + 1) * 128],
                            ident,
                        )
                    xt_sb = xtpool.tile([128, GRP], F16)
                    nc.scalar.copy(xt_sb, xt_ps)
                    o_ps = pso.tile([128, GRP], F32, tag="o_ps")
                    for b in range(nblk):
                        nc.tensor.matmul(
                            o_ps[:, b * 128:(b + 1) * 128],
                            xt_sb[:, b * 128:(b + 1) * 128],
                            t2_s,
                        )
                    nc.vector.tensor_add(
                        otile[:, g * GRP:(g + 1) * GRP], o_ps, bb_s
                    )
                # Stores issue from their own engine queue so a store
                # waiting on compute can't head-of-line-block the next
                # chunk load behind it in the SP sequencer.
                if store_engine == "gpsimd":
                    nc.gpsimd.dma_start(o_view[c], otile)
                elif store_engine == "act":
                    nc.scalar.dma_start(o_view[c], otile)
                elif store_engine == "vector":
                    nc.vector.dma_start(o_view[c], otile)
                else:
                    nc.sync.dma_start(o_view[c], otile)
    nc.compile()
    return nc


def _get_runner():
    """Build (once) a jitted shard_map callable over the 8-core mesh."""
    if "runner" in _cache:
        return _cache["runner"]

    import jax
    from jax.sharding import Mesh, PartitionSpec
    from jax.experimental.shard_map import shard_map
    import concourse.mybir as mybir
    from concourse import bass2jax

    bass2jax.install_neuronx_cc_hook()
    nc = _build_nc()
    _cache["nc"] = nc

    partition_name = (
        nc.partition_id_tensor.name if nc.partition_id_tensor else None
    )
    in_names = []
    out_names = []
    out_avals = []
    for alloc in nc.m.functions[0].allocations:
        if not isinstance(alloc, mybir.MemoryLocationSet):
            continue
        name = alloc.memorylocations[0].name
        if alloc.kind == "ExternalInput":
            if name != partition_name:
                in_names.append(name)
        elif alloc.kind == "ExternalOutput":
            out_names.append(name)
            out_avals.append(
                jax.core.ShapedArray(
                    tuple(alloc.tensor_shape), mybir.dt.np(alloc.dtype)
                )
            )
    n_params = len(in_names)
    all_in_names = in_names + out_names
    if partition_name is not None:
        all_in_names = all_in_names + [partition_name]

    def _body(*args):
        operands = list(args)
        if partition_name is not None:
            operands.append(bass2jax.partition_id_tensor())
        outs = bass2jax._bass_exec_p.bind(
            *operands,
            out_avals=tuple(out_avals),
            in_names=tuple(all_in_names),
            out_names=tuple(out_names),
            lowering_input_output_aliases=(),
            sim_require_finite=True,
            sim_require_nnan=True,
            nc=nc,
        )
        return tuple(outs)

    devices = jax.devices()[:_NCORES]
    mesh = Mesh(np.asarray(devices), ("core",))
    spec = PartitionSpec("core")
    n_outs = len(out_names)
    # No donation: the kernel writes every output element, so the dummy
    # output operands (required by the bass custom call's parameter
    # list) can be one cached device-resident zeros array reused across
    # calls instead of a fresh 64MB host->device transfer per call.
    fn = jax.jit(
        shard_map(
            _body,
            mesh=mesh,
            in_specs=(spec,) * (n_params + n_outs),
            out_specs=(spec,) * n_outs,
            check_rep=False,
        ),
        keep_unused=True,
    )
    _cache["runner"] = (fn, in_names, out_names, mesh, spec)
    return _cache["runner"]


def _prepare_inputs(x, weight_left, weight_right, bias):
    """Host-side prep: shard x, fold weights, broadcast bias."""
    T2 = _fold_weights(weight_left, weight_right)
    bias_flat = np.ascontiguousarray(bias, dtype=np.float32).reshape(_O * _DIM)
    BB4 = np.tile(
        np.broadcast_to(bias_flat, (128, 128)), (1, _GRP // 128)
    ).astype(np.float32)
    x_flat = np.ascontiguousarray(x, dtype=np.float32).reshape(_NTOK, 128)
    x_flat = x_flat.astype(np.float16)
    # global concat layout for shard_map: inputs stacked along axis 0
    ins = {
        "xs": x_flat,                                   # [NTOK, 128] fp16
        "t2": np.tile(T2, (_NCORES, 1)),                # replicate per core
        "idn": np.tile(np.eye(128, dtype=np.float16), (_NCORES, 1)),
        "bb4": np.tile(BB4, (_NCORES, 1)),
    }
    return ins


def _out_dummy():
    """Cached device-resident dummy for the output operand slot."""
    import jax
    from jax.sharding import NamedSharding

    if "zeros" not in _cache:
        fn, in_names, out_names, mesh, spec = _get_runner()
        sharding = NamedSharding(mesh, spec)
        z = jax.device_put(np.zeros((_NTOK, 128), np.float16), sharding)
        z.block_until_ready()
        _cache["zeros"] = z
    return _cache["zeros"]


def _run_device(ins):
    import jax
    from jax.sharding import NamedSharding

    fn, in_names, out_names, mesh, spec = _get_runner()
    sharding = NamedSharding(mesh, spec)
    args = [jax.device_put(ins[n], sharding) for n in in_names]
    outs = fn(*args, _out_dummy())
    return np.asarray(outs[0])


def kernel(x, weight_left, weight_right, bias):
    x = np.asarray(x)
    weight_left = np.asarray(weight_left)
    weight_right = np.asarray(weight_right)
    bias = np.asarray(bias)
    ins = _prepare_inputs(x, weight_left, weight_right, bias)
    out_flat = _run_device(ins)
    return out_flat.astype(np.float32).reshape(_B, _S, _O, _DIM)


def _bench_args():
    """Device-resident inputs for timing runs."""
    import jax
    from jax.sharding import NamedSharding

    rng = np.random.default_rng(0)
    x = rng.standard_normal((_B, _S, _I, _DIM), dtype=np.float32)
    wl = (rng.standard_normal((_K, _O, _I, _DIM)) * 0.02).astype(np.float32)
    wr = (rng.standard_normal((_K, _O, _I, _DIM)) * 0.02).astype(np.float32)
    bias = np.zeros((_O, _DIM), np.float32)
    ins = _prepare_inputs(x, wl, wr, bias)

    fn, in_names, out_names, mesh, spec = _get_runner()
    sharding = NamedSharding(mesh, spec)
    args = [jax.device_put(ins[n], sharding) for n in in_names]
    return fn, args


def _ntff_profile_hook():
    """NTFF profiling via ctypes into libaxon_pjrt.so (the NRT profile
    sidechannel). Returns a contextmanager factory or None if absent."""
    import contextlib
    import ctypes

    so_path = "/opt/axon/libaxon_pjrt.so"
    try:
        lib = ctypes.CDLL(so_path)
        lib.axon_start_nrt_profile
    except (OSError, AttributeError):
        return None
    lib.axon_start_nrt_profile.argtypes = [
        ctypes.POINTER(ctypes.c_int64), ctypes.c_size_t,
    ]
    lib.axon_start_nrt_profile.restype = ctypes.c_int64
    lib.axon_stop_nrt_profile.argtypes = [ctypes.c_char_p]
    lib.axon_stop_nrt_profile.restype = ctypes.c_int64

    @contextlib.contextmanager
    def _hook(output_dir, device_ids):
        import jax

        jax.devices()
        ids = (ctypes.c_int64 * len(device_ids))(*device_ids)
        rc = lib.axon_start_nrt_profile(ids, len(device_ids))
        if rc != 0:
            raise RuntimeError(f"axon_start_nrt_profile rc={rc}")
        try:
            yield
        finally:
            n = lib.axon_stop_nrt_profile(str(output_dir).encode())
            if n <= 0:
                raise RuntimeError(f"axon_stop_nrt_profile wrote {n} files")

    return _hook


def _hw_exec_time_profiled(n_reps=3):
    """True on-device execution time via neuron-profile (NTFF capture on
    all 8 cores; per capture take the max across cores = the parallel
    HW execution time; report the median across captures)."""
    import os
    import shutil
    import statistics

    hook = _ntff_profile_hook()
    if hook is None:
        return None
    from concourse._compat import FishPath
    import gauge.profiler

    fn, args = _bench_args()
    z = _out_dummy()
    fn(*args, z)[0].block_until_ready()  # warm

    nc = _cache["nc"]
    cores = list(range(_NCORES))
    per_capture_max = []
    for rep in range(n_reps):
        out_dir = f"/tmp/kernel_ntff_{os.getpid()}_{rep}"
        shutil.rmtree(out_dir, ignore_errors=True)
        os.makedirs(out_dir, exist_ok=True)
        with hook(out_dir, cores):
            fn(*args, z)[0].block_until_ready()
        profile = gauge.profiler.Profile(
            profile_path=FishPath(out_dir),
            kernel_dev_mode=True,
            profile_on_exit=False,
            bass_kernel=nc.m,
            offline_processing=True,
            fname="*_body*",
        )
        results = profile.to_perfetto(model_index=tuple(cores))
        times = [r.exec_time_ns for r in results if r.exec_time_ns]
        if not times:
            return None
        per_capture_max.append(max(times))
        shutil.rmtree(out_dir, ignore_errors=True)
    return statistics.median(per_capture_max)


def _timed_run(n_iters=30):
    """HW execution time in ns. Prefers the neuron-profile (NTFF) number
    — the true on-device time; falls back to host wall-clock around the
    device-resident-input launch (which on axon-tunneled cores is
    dominated by dispatch latency) if profiling is unavailable."""
    try:
        ns = _hw_exec_time_profiled()
        if ns is not None:
            return ns
    except Exception as e:  # noqa: BLE001 -- any profiling failure
        import sys as _sys

        print(f"ntff profiling unavailable ({e!r}); wall-clock fallback",
              file=_sys.stderr)

    import time

    fn, args = _bench_args()
    z = _out_dummy()
    fn(*args, z)[0].block_until_ready()  # compile+warm
    best = float("inf")
    for _ in range(n_iters):
        t0 = time.perf_counter()
        out = fn(*args, z)
        out[0].block_until_ready()
        best = min(best, time.perf_counter() - t0)
    return best * 1e9


if __name__ == "__main__":
    ns = _timed_run()
    print(f"HW exec time: {ns:.0f} ns")
